# revision 10
# baseline (speedup 1.0000x reference)
"""KGAttentionLayer Trainium2 kernel.

Sharding: 8 cores = (batch 2) x (query-block 4). Core c handles batch
b=c//4, query rows [j*512, (j+1)*512) of that batch (j=c%4). Each core
computes k/v/kg projections for its whole batch (duplicated within the
4-core batch group — cheaper than cross-core collectives on this
topology), attention for its 512 queries over all 16 heads, and the
gate/out-proj/residual for its rows. No collectives.

Layouts (everything pre-transposed on host so the PE never needs an
on-device transpose):
  xT      [1024, 2048]  x[b].T, rolled so the core's query block is at
                        columns [0, 512)
  scoresT [m, l] chunks via  lhsT=kT[64, m128], rhs=qT[64, l512]
  v_pad   [2304, 16*65]  rows m, per-head 64 v-cols (+bias baked) and a
          ones column -> the attn@v matmul emits the softmax numerator in
          PSUM rows 0..63 and the denominator in row 64 of the same tile.
"""

import sys

sys.path.insert(0, "/opt/trn_rl_repo")

import numpy as np

import concourse.bass as bass
import concourse.mybir as mybir
import concourse.tile as tile
from concourse import bacc
from concourse.bass_utils import run_bass_kernel_spmd

F32 = mybir.dt.float32
F32R = mybir.dt.float32r
AF = mybir.ActivationFunctionType

D = 1024
H = 16
HD = 64
B = 2
L = 2048
E = 256
LBLK = 512          # queries per core
M = L + E           # 2304 attended positions
NMC = M // 128      # 18 m-chunks
N_CORES = 8

_CACHE = {}


def _build():
    nc = bacc.Bacc("TRN2", target_bir_lowering=False, debug=False,
                   num_devices=N_CORES)

    dram = {}

    def din(name, shape, dt=F32R):
        dram[name] = nc.dram_tensor(name, shape, dt, kind="ExternalInput")
        return dram[name]

    xT = din("xT", [D, L])
    kgT = din("kgT", [D, E])
    WqT = din("WqT", [D, D])
    WkT = din("WkT", [D, D])
    WkkT = din("WkkT", [D, D])
    WvT = din("WvT", [D, D])
    WkvT = din("WkvT", [D, D])
    WoT = din("WoT", [D, D])
    WgT = din("WgT", [D, D])
    bq = din("bq", [128, 8], F32)      # col g = (bias*0.125)[g*128:(g+1)*128]
    bk = din("bk", [128, 8], F32)
    bkk = din("bkk", [128, 8], F32)
    bo = din("bo", [128, 8], F32)
    bge = din("bge", [128, 8], F32)    # bg + Wg[:,1024:] @ kg_mean(batch)
    bvb = din("bvb", [128, D], F32)    # np.tile(bv, (128,1))
    bkvb = din("bkvb", [128, D], F32)

    OUTT = nc.dram_tensor("OUTT", [D, LBLK], F32, kind="ExternalOutput")

    # internal DRAM intermediates
    kTa = nc.dram_tensor("kTa", [D, M], F32R)          # k_aug transposed
    vpad = nc.dram_tensor("vpad", [M, H * 65], F32R)   # [m, head*(64 v + one)]

    def w8(pool, W, g, tag="w8"):
        t = pool.tile([128, 8, 128], F32R, tag=tag)
        nc.sync.dma_start(
            t[:], W.ap().rearrange("(kk p) d -> p kk d", p=128)
            [:, :, g * 128:(g + 1) * 128])
        return t

    from contextlib import ExitStack

    with tile.TileContext(nc) as tc, ExitStack() as ctx:
        persist = ctx.enter_context(tc.tile_pool(name="persist", bufs=1))
        wpool = ctx.enter_context(tc.tile_pool(name="wpool", bufs=2))
        spool = ctx.enter_context(tc.tile_pool(name="spool", bufs=2))
        epool = ctx.enter_context(tc.tile_pool(name="epool", bufs=3))
        kpool = ctx.enter_context(tc.tile_pool(name="kpool", bufs=6))
        vpool = ctx.enter_context(tc.tile_pool(name="vpool", bufs=2))
        psA = ctx.enter_context(tc.tile_pool(name="psA", bufs=3, space="PSUM"))
        psS = ctx.enter_context(tc.tile_pool(name="psS", bufs=2, space="PSUM"))
        psV = ctx.enter_context(tc.tile_pool(name="psV", bufs=2, space="PSUM"))
        psR = ctx.enter_context(tc.tile_pool(name="psR", bufs=1, space="PSUM"))

        if True:
            # ---- resident loads ----
            xts = persist.tile([128, 8, L], F32R, tag="xts")
            nc.sync.dma_start(
                xts[:], xT.ap().rearrange("(kk p) l -> p kk l", p=128))
            kgts = persist.tile([128, 8, E], F32R, tag="kgts")
            nc.sync.dma_start(
                kgts[:], kgT.ap().rearrange("(kk p) e -> p kk e", p=128))
            biases = {}
            for nm in ("bq", "bk", "bkk", "bo", "bge"):
                t = persist.tile([128, 8], F32, tag=nm)
                nc.sync.dma_start(t[:], dram[nm].ap())
                biases[nm] = t
            bvbs = persist.tile([128, D], F32, tag="bvbs")
            nc.sync.dma_start(bvbs[:], bvb.ap())
            bkvbs = persist.tile([128, D], F32, tag="bkvbs")
            nc.sync.dma_start(bkvbs[:], bkvb.ap())
            ones16 = persist.tile([128, 16, 1], F32, tag="ones16")
            nc.vector.memset(ones16[:], 1.0)
            ones1 = persist.tile([1, 64], F32, tag="ones1")
            nc.vector.memset(ones1[:], 1.0)

            qts = persist.tile([64, H, LBLK], F32R, tag="qts")
            outTs = persist.tile([128, 8, LBLK], F32R, tag="outTs")

            # ---- phase A: projections ----
            # q: only columns [0, 512) of (rolled) xT
            for g in range(8):
                wq = w8(wpool, WqT, g)
                ps = psA.tile([128, LBLK], F32, tag="psA")
                for kk in range(8):
                    nc.tensor.matmul(ps[:], wq[:, kk, :], xts[:, kk, 0:LBLK],
                                     start=(kk == 0), stop=(kk == 7))
                nc.scalar.activation(qts[:, 2 * g, :], ps[0:64, :], AF.Identity,
                                     bias=biases["bq"][0:64, g:g + 1], scale=0.125)
                nc.scalar.activation(qts[:, 2 * g + 1, :], ps[64:128, :],
                                     AF.Identity,
                                     bias=biases["bq"][64:128, g:g + 1],
                                     scale=0.125)

            # k over all L, then kg cols
            for g in range(8):
                wk = w8(wpool, WkT, g)
                for lc in range(4):
                    ps = psA.tile([128, 512], F32, tag="psA")
                    for kk in range(8):
                        nc.tensor.matmul(
                            ps[:], wk[:, kk, :], xts[:, kk, lc * 512:(lc + 1) * 512],
                            start=(kk == 0), stop=(kk == 7))
                    kt = spool.tile([128, 512], F32R, tag="ev")
                    nc.scalar.activation(kt[:], ps[:], AF.Identity,
                                         bias=biases["bk"][:, g:g + 1])
                    nc.sync.dma_start(
                        kTa.ap()[g * 128:(g + 1) * 128, lc * 512:(lc + 1) * 512],
                        kt[:])
                wkk = w8(wpool, WkkT, g)
                ps = psA.tile([128, E], F32, tag="psA")
                for kk in range(8):
                    nc.tensor.matmul(ps[:], wkk[:, kk, :], kgts[:, kk, :],
                                     start=(kk == 0), stop=(kk == 7))
                kt = spool.tile([128, E], F32R, tag="evkg")
                nc.scalar.activation(kt[:], ps[:], AF.Identity,
                                     bias=biases["bkk"][:, g:g + 1])
                nc.sync.dma_start(kTa.ap()[g * 128:(g + 1) * 128, L:M], kt[:])

            # v_pad rows: x part (mc 0..15), kg part (mc 16..17)
            for dc in range(4):
                dlo = dc * 256
                wv = wpool.tile([128, 8, 256], F32R, tag="wv")
                nc.sync.dma_start(
                    wv[:], WvT.ap().rearrange("(kk p) d -> p kk d", p=128)
                    [:, :, dlo:dlo + 256])
                for mc in range(16):
                    ps = psV.tile([128, 256], F32, tag="psV")
                    for kk in range(8):
                        nc.tensor.matmul(
                            ps[:], xts[:, kk, mc * 128:(mc + 1) * 128],
                            wv[:, kk, :], start=(kk == 0), stop=(kk == 7))
                    vs = spool.tile([128, 256], F32R, tag="ev")
                    nc.vector.tensor_add(vs[:], ps[:], bvbs[:, dlo:dlo + 256])
                    nc.sync.dma_start(
                        vpad.ap()[mc * 128:(mc + 1) * 128, :]
                        .rearrange("p (h c) -> p h c", c=65)
                        [:, dc * 4:(dc + 1) * 4, 0:64],
                        vs[:].rearrange("p (h c) -> p h c", c=64))
                wkv = wpool.tile([128, 8, 256], F32R, tag="wv")
                nc.sync.dma_start(
                    wkv[:], WkvT.ap().rearrange("(kk p) d -> p kk d", p=128)
                    [:, :, dlo:dlo + 256])
                for mc in range(2):
                    ps = psV.tile([128, 256], F32, tag="psV")
                    for kk in range(8):
                        nc.tensor.matmul(
                            ps[:], kgts[:, kk, mc * 128:(mc + 1) * 128],
                            wkv[:, kk, :], start=(kk == 0), stop=(kk == 7))
                    vs = spool.tile([128, 256], F32R, tag="ev")
                    nc.vector.tensor_add(vs[:], ps[:], bkvbs[:, dlo:dlo + 256])
                    nc.sync.dma_start(
                        vpad.ap()[L + mc * 128:L + (mc + 1) * 128, :]
                        .rearrange("p (h c) -> p h c", c=65)
                        [:, dc * 4:(dc + 1) * 4, 0:64],
                        vs[:].rearrange("p (h c) -> p h c", c=64))
            for mc in range(NMC):
                nc.sync.dma_start(
                    vpad.ap()[mc * 128:(mc + 1) * 128, :]
                    .rearrange("p (h c) -> p h c", c=65)[:, :, 64:65],
                    ones16[:].bitcast(F32R))

            # ---- phase B: attention per head ----
            for h in range(H):
                g, po = h // 2, (h % 2) * 64
                vph_t = vpool.tile([128, NMC, 65], F32R, tag="vph")
                nc.sync.dma_start(
                    vph_t[:],
                    vpad.ap().rearrange("(mc p) c -> p mc c", p=128)
                    [:, :, 65 * h:65 * (h + 1)])
                av = psV.tile([65, LBLK], F32, tag="psV")
                for mc in range(NMC):
                    kth = kpool.tile([64, 128], F32R, tag="kth")
                    nc.sync.dma_start(
                        kth[:],
                        kTa.ap()[h * 64:(h + 1) * 64, mc * 128:(mc + 1) * 128])
                    sp = psS.tile([128, LBLK], F32, tag="psS")
                    nc.tensor.matmul(sp[:], kth[:], qts[:, h, :],
                                     start=True, stop=True)
                    et = epool.tile([128, LBLK], F32R, tag="et")
                    nc.scalar.activation(et[:], sp[:], AF.Exp)
                    nc.tensor.matmul(av[:], vph_t[:, mc, :], et[:],
                                     start=(mc == 0), stop=(mc == NMC - 1))
                rec = spool.tile([1, LBLK], F32, tag="rec")
                nc.vector.reciprocal(rec[:], av[64:65, :])
                recr = spool.tile([1, LBLK], F32R, tag="recr")
                nc.scalar.activation(recr[:], rec[:], AF.Identity)
                rp = psR.tile([64, LBLK], F32, tag="psR")
                nc.tensor.matmul(rp[:], ones1[:].bitcast(F32R), recr[:],
                                 start=True, stop=True)
                avs = spool.tile([64, LBLK], F32, tag="avs")
                nc.scalar.activation(avs[:], av[0:64, :], AF.Identity)
                nc.vector.tensor_mul(outTs[po:po + 64, g, :], avs[:], rp[:])

            # ---- phase C: out-proj + gate + residual ----
            for g in range(8):
                wo = w8(wpool, WoT, g)
                wg = w8(wpool, WgT, g)
                pp = psA.tile([128, LBLK], F32, tag="psA")
                for kk in range(8):
                    nc.tensor.matmul(pp[:], wo[:, kk, :], outTs[:, kk, :],
                                     start=(kk == 0), stop=(kk == 7))
                pj = spool.tile([128, LBLK], F32, tag="pj")
                nc.scalar.activation(pj[:], pp[:], AF.Identity,
                                     bias=biases["bo"][:, g:g + 1])
                gp = psA.tile([128, LBLK], F32, tag="psA")
                for kk in range(8):
                    nc.tensor.matmul(gp[:], wg[:, kk, :], outTs[:, kk, :],
                                     start=(kk == 0), stop=(kk == 7))
                gt = spool.tile([128, LBLK], F32, tag="gt")
                nc.scalar.activation(gt[:], gp[:], AF.Sigmoid,
                                     bias=biases["bge"][:, g:g + 1])
                xs = xts[:, g, 0:LBLK].bitcast(F32)
                d1 = spool.tile([128, LBLK], F32, tag="fin")
                nc.vector.tensor_sub(d1[:], pj[:], xs)
                d2 = spool.tile([128, LBLK], F32, tag="fin")
                nc.vector.tensor_mul(d2[:], d1[:], gt[:])
                fo = spool.tile([128, LBLK], F32, tag="fin")
                nc.vector.tensor_add(fo[:], d2[:], xs)
                nc.sync.dma_start(OUTT.ap()[g * 128:(g + 1) * 128, :], fo[:])

    nc.compile()
    return nc


def kernel(x, kg_embeds, Wq, bq, Wk, bk, Wv, bv, Wkk, bkk, Wkv, bkv,
           Wo, bo, Wg, bg):
    x = np.asarray(x, np.float32)
    kg_embeds = np.asarray(kg_embeds, np.float32)
    ws = {k: np.asarray(v, np.float32) for k, v in dict(
        Wq=Wq, bq=bq, Wk=Wk, bk=bk, Wv=Wv, bv=bv, Wkk=Wkk, bkk=bkk,
        Wkv=Wkv, bkv=bkv, Wo=Wo, bo=bo, Wg=Wg, bg=bg).items()}

    if "nc" not in _CACHE:
        _CACHE["nc"] = _build()
    nc = _CACHE["nc"]

    def col8(v):
        return np.ascontiguousarray(v.reshape(8, 128).T)

    shared = {
        "WqT": np.ascontiguousarray(ws["Wq"].T),
        "WkT": np.ascontiguousarray(ws["Wk"].T),
        "WkkT": np.ascontiguousarray(ws["Wkk"].T),
        "WvT": np.ascontiguousarray(ws["Wv"].T),
        "WkvT": np.ascontiguousarray(ws["Wkv"].T),
        "WoT": np.ascontiguousarray(ws["Wo"].T),
        "WgT": np.ascontiguousarray(ws["Wg"][:, :D].T),
        "bq": col8(ws["bq"] * 0.125),
        "bk": col8(ws["bk"]),
        "bkk": col8(ws["bkk"]),
        "bo": col8(ws["bo"]),
        "bvb": np.ascontiguousarray(np.tile(ws["bv"], (128, 1))),
        "bkvb": np.ascontiguousarray(np.tile(ws["bkv"], (128, 1))),
    }

    in_maps = []
    for c in range(N_CORES):
        b, j = divmod(c, 4)
        # roll the core's query block to columns [0, 512); k/v attend over
        # all columns, so their (rolled) order is irrelevant to softmax
        xb = np.ascontiguousarray(np.roll(x[b].T, -j * LBLK, axis=1))
        kgm = kg_embeds[b].mean(axis=0)
        bge = ws["bg"] + ws["Wg"][:, D:] @ kgm
        m = dict(shared)
        m["xT"] = xb
        m["kgT"] = np.ascontiguousarray(kg_embeds[b].T)
        m["bge"] = col8(bge)
        in_maps.append(m)

    _CACHE["in_maps"] = in_maps
    res = run_bass_kernel_spmd(nc, in_maps, core_ids=list(range(N_CORES)))
    out = np.empty((B, L, D), np.float32)
    for c in range(N_CORES):
        b, j = divmod(c, 4)
        out[b, j * LBLK:(j + 1) * LBLK, :] = res.results[c]["OUTT"].T
    return out


# revision 11
# speedup vs baseline: 1.7613x; 1.7613x over previous
"""KGAttentionLayer Trainium2 kernel (v2: bf16 matmuls, SBUF-resident,
head-block interleaved to keep the PE dense and the HAM clock warm).

Sharding: 8 cores = (batch 2) x (query-block 4). Core c handles batch
b=c//4, query rows [j*512, (j+1)*512) of that batch (j=c%4). Each core
computes k/v/kg projections for its whole batch (duplicated within the
4-core batch group — cheaper than collectives on this topology),
attention for its 512 queries over all 16 heads, and the gate/out-proj/
residual for its rows. No collectives, no DRAM intermediates.

The work is emitted in 4 blocks of 4 heads; each block projects the
q/k/v/kg slices those heads need and immediately runs their attention.
Projection matmuls fill the PE while the scalar engine works through
the exp() of the previous heads, so the PE never idles long enough for
the HAM clock gate to drop to 1.2 GHz.

Layouts (host pre-transposes; the PE never transposes on device):
  xT      [1024, 2048]  x[b].T, rolled so the core's query block is at
                        columns [0, 512)
  scoresT [m, l] chunks via  lhsT=kT[64, m128], rhs=qT[64, l512]
  vslab   [128, 18, 4*65] per-head 64 v-cols (+bias baked in) plus a
          ones column -> the attn@v matmul emits the softmax numerator
          in PSUM rows 0..63 and the denominator in row 64.
"""

import sys

sys.path.insert(0, "/opt/trn_rl_repo")

import numpy as np

import concourse.bass as bass
import concourse.mybir as mybir
import concourse.tile as tile
from concourse import bacc
from concourse.bass_utils import run_bass_kernel_spmd

F32 = mybir.dt.float32
BF16 = mybir.dt.bfloat16
AF = mybir.ActivationFunctionType
OP = mybir.AluOpType

D = 1024
H = 16
HD = 64
B = 2
L = 2048
E = 256
LBLK = 512          # queries per core
M = L + E           # 2304 attended positions
NMC = M // 128      # 18 m-chunks
N_CORES = 8

_CACHE = {}


def _build():
    nc = bacc.Bacc("TRN2", target_bir_lowering=False, debug=False,
                   num_devices=N_CORES)

    dram = {}

    def din(name, shape, dt=BF16):
        dram[name] = nc.dram_tensor(name, shape, dt, kind="ExternalInput")
        return dram[name]

    xT = din("xT", [D, L])
    xres = din("xres", [128, 8, LBLK], F32)
    kgT = din("kgT", [D, E])
    WqT = din("WqT", [D, D])
    WkT = din("WkT", [D, D])
    WkkT = din("WkkT", [D, D])
    WvT = din("WvT", [D, D])
    WkvT = din("WkvT", [D, D])
    WoT = din("WoT", [D, D])
    WgT = din("WgT", [D, D])
    bq = din("bq", [128, 8], F32)      # col g = (bias*0.125)[g*128:(g+1)*128]
    bk = din("bk", [128, 8], F32)
    bkk = din("bkk", [128, 8], F32)
    bo = din("bo", [128, 8], F32)
    bge = din("bge", [128, 8], F32)    # bg + Wg[:,1024:] @ kg_mean(batch)
    bvb = din("bvb", [128, D], F32)    # np.tile(bv, (128,1))
    bkvb = din("bkvb", [128, D], F32)

    OUTT = nc.dram_tensor("OUTT", [D, LBLK], F32, kind="ExternalOutput")

    def w8(pool, W, g, tag="w8"):
        t = pool.tile([128, 8, 128], BF16, tag=tag, name=f"{tag}_{W.name}_{g}")
        nc.sync.dma_start(
            t[:], W.ap().rearrange("(kk p) d -> p kk d", p=128)
            [:, :, g * 128:(g + 1) * 128])
        return t

    from contextlib import ExitStack

    with tile.TileContext(nc) as tc, ExitStack() as ctx:
        persist = ctx.enter_context(tc.tile_pool(name="persist", bufs=1))
        wpool = ctx.enter_context(tc.tile_pool(name="wpool", bufs=2))
        spool = ctx.enter_context(tc.tile_pool(name="spool", bufs=2))
        epool = ctx.enter_context(tc.tile_pool(name="epool", bufs=3))
        blkpool = ctx.enter_context(tc.tile_pool(name="blkpool", bufs=2))
        psA = ctx.enter_context(tc.tile_pool(name="psA", bufs=2, space="PSUM"))
        psV = ctx.enter_context(tc.tile_pool(name="psV", bufs=2, space="PSUM"))
        psS = ctx.enter_context(tc.tile_pool(name="psS", bufs=2, space="PSUM"))
        psAV = ctx.enter_context(tc.tile_pool(name="psAV", bufs=1, space="PSUM"))
        psR = ctx.enter_context(tc.tile_pool(name="psR", bufs=1, space="PSUM"))

        # ---- resident loads ----
        xts = persist.tile([128, 8, L], BF16, tag="xts")
        nc.sync.dma_start(xts[:], xT.ap().rearrange("(kk p) l -> p kk l", p=128))
        xrs = persist.tile([128, 8, LBLK], F32, tag="xrs")
        nc.sync.dma_start(xrs[:], xres.ap())
        kgts = persist.tile([128, 8, E], BF16, tag="kgts")
        nc.sync.dma_start(kgts[:], kgT.ap().rearrange("(kk p) e -> p kk e", p=128))
        biases = {}
        for nm in ("bq", "bk", "bkk", "bo", "bge"):
            t = persist.tile([128, 8], F32, tag=nm, name=nm + "_sb")
            nc.sync.dma_start(t[:], dram[nm].ap())
            biases[nm] = t
        bvbs = persist.tile([128, D], F32, tag="bvbs")
        nc.sync.dma_start(bvbs[:], bvb.ap())
        bkvbs = persist.tile([128, D], F32, tag="bkvbs")
        nc.sync.dma_start(bkvbs[:], bkvb.ap())
        onesv = persist.tile([128, NMC, 4, 1], BF16, tag="onesv")
        nc.vector.memset(onesv[:], 1.0)
        ones1 = persist.tile([1, 64], BF16, tag="ones1")
        nc.vector.memset(ones1[:], 1.0)

        qts = persist.tile([64, H, LBLK], BF16, tag="qts")
        outTs = persist.tile([128, 8, LBLK], BF16, tag="outTs")

        for blk in range(4):
            g0 = 2 * blk
            kta = blkpool.tile([64, 4, M], BF16, tag="kta")
            vslab = blkpool.tile([128, NMC, 4 * 65], BF16, tag="vslab")
            # ones columns for the whole slab in one strided copy
            nc.vector.tensor_copy(
                vslab[:].rearrange("p mc (h c) -> p mc h c", c=65)[:, :, :, 64:65],
                onesv[:])

            # ---- q projection (heads 4blk..4blk+3) ----
            for gi, g in enumerate((g0, g0 + 1)):
                wq = w8(wpool, WqT, g)
                ps = psA.tile([128, LBLK], F32, tag="psA", name=f"qp{g}")
                for kk in range(8):
                    nc.tensor.matmul(ps[:], wq[:, kk, :], xts[:, kk, 0:LBLK],
                                     start=(kk == 0), stop=(kk == 7))
                nc.vector.tensor_scalar(
                    qts[:, 2 * g, :], ps[0:64, :], 0.125,
                    biases["bq"][0:64, g:g + 1], OP.mult, OP.add)
                nc.vector.tensor_scalar(
                    qts[:, 2 * g + 1, :], ps[64:128, :], 0.125,
                    biases["bq"][64:128, g:g + 1], OP.mult, OP.add)

            # ---- k projection ----
            for gi, g in enumerate((g0, g0 + 1)):
                wk = w8(wpool, WkT, g)
                for lc in range(4):
                    ps = psA.tile([128, 512], F32, tag="psA", name=f"kp{g}_{lc}")
                    for kk in range(8):
                        nc.tensor.matmul(
                            ps[:], wk[:, kk, :],
                            xts[:, kk, lc * 512:(lc + 1) * 512],
                            start=(kk == 0), stop=(kk == 7))
                    sl = slice(lc * 512, (lc + 1) * 512)
                    nc.vector.tensor_scalar_add(
                        kta[:, 2 * gi, sl], ps[0:64, :],
                        biases["bk"][0:64, g:g + 1])
                    nc.vector.tensor_scalar_add(
                        kta[:, 2 * gi + 1, sl], ps[64:128, :],
                        biases["bk"][64:128, g:g + 1])
                wkk = w8(wpool, WkkT, g)
                ps = psA.tile([128, E], F32, tag="psA", name=f"kkp{g}")
                for kk in range(8):
                    nc.tensor.matmul(ps[:], wkk[:, kk, :], kgts[:, kk, :],
                                     start=(kk == 0), stop=(kk == 7))
                nc.vector.tensor_scalar_add(
                    kta[:, 2 * gi, L:M], ps[0:64, :],
                    biases["bkk"][0:64, g:g + 1])
                nc.vector.tensor_scalar_add(
                    kta[:, 2 * gi + 1, L:M], ps[64:128, :],
                    biases["bkk"][64:128, g:g + 1])

            # ---- v projection (dv columns [256*blk, 256*blk+256)) ----
            dlo = 256 * blk
            wv = wpool.tile([128, 8, 256], BF16, tag="wv", name=f"wv{blk}")
            nc.sync.dma_start(
                wv[:], WvT.ap().rearrange("(kk p) d -> p kk d", p=128)
                [:, :, dlo:dlo + 256])
            wkv = wpool.tile([128, 8, 256], BF16, tag="wv", name=f"wkv{blk}")
            nc.sync.dma_start(
                wkv[:], WkvT.ap().rearrange("(kk p) d -> p kk d", p=128)
                [:, :, dlo:dlo + 256])
            for mc in range(NMC):
                ps = psV.tile([128, 256], F32, tag="psV", name=f"vp{blk}_{mc}")
                for kk in range(8):
                    if mc < 16:
                        lhsT = xts[:, kk, mc * 128:(mc + 1) * 128]
                    else:
                        lhsT = kgts[:, kk, (mc - 16) * 128:(mc - 15) * 128]
                    nc.tensor.matmul(ps[:], lhsT, (wv if mc < 16 else wkv)[:, kk, :],
                                     start=(kk == 0), stop=(kk == 7))
                bb = bvbs if mc < 16 else bkvbs
                nc.vector.tensor_add(
                    vslab[:, mc, :].rearrange("p (h c) -> p h c", c=65)
                    [:, :, 0:64],
                    ps[:].rearrange("p (h c) -> p h c", c=64),
                    bb[:, dlo:dlo + 256].rearrange("p (h c) -> p h c", c=64))

            # ---- attention for heads 4blk..4blk+3 ----
            for hh in range(4):
                h = 4 * blk + hh
                avp = psAV.tile([65, LBLK], F32, tag="psAV", name=f"av{h}")
                for mc in range(NMC):
                    sp = psS.tile([128, LBLK], F32, tag="psS", name=f"sp{h}_{mc}")
                    nc.tensor.matmul(sp[:], kta[:, hh, mc * 128:(mc + 1) * 128],
                                     qts[:, h, :], start=True, stop=True)
                    et = epool.tile([128, LBLK], BF16, tag="et", name=f"et{h}_{mc}")
                    nc.scalar.activation(et[:], sp[:], AF.Exp)
                    nc.tensor.matmul(avp[:], vslab[:, mc, hh * 65:hh * 65 + 65],
                                     et[:], start=(mc == 0), stop=(mc == NMC - 1))
                rec = spool.tile([1, LBLK], F32, tag="rec", name=f"rec{h}")
                nc.vector.reciprocal(rec[:], avp[64:65, :])
                recr = spool.tile([1, LBLK], BF16, tag="recr", name=f"recr{h}")
                nc.scalar.activation(recr[:], rec[:], AF.Identity)
                rp = psR.tile([64, LBLK], F32, tag="psR", name=f"rp{h}")
                nc.tensor.matmul(rp[:], ones1[:], recr[:], start=True, stop=True)
                avs = spool.tile([64, LBLK], F32, tag="avs", name=f"avs{h}")
                nc.scalar.activation(avs[:], avp[0:64, :], AF.Identity)
                nc.vector.tensor_mul(outTs[(h % 2) * 64:(h % 2) * 64 + 64,
                                           h // 2, :], avs[:], rp[:])

        # ---- out-proj + gate + residual ----
        for g in range(8):
            wo = w8(wpool, WoT, g)
            wg = w8(wpool, WgT, g)
            pp = psA.tile([128, LBLK], F32, tag="psA", name=f"pp{g}")
            for kk in range(8):
                nc.tensor.matmul(pp[:], wo[:, kk, :], outTs[:, kk, :],
                                 start=(kk == 0), stop=(kk == 7))
            pj = spool.tile([128, LBLK], F32, tag="pj", name=f"pj{g}")
            nc.vector.tensor_scalar_add(pj[:], pp[:], biases["bo"][:, g:g + 1])
            gp = psA.tile([128, LBLK], F32, tag="psA", name=f"gp{g}")
            for kk in range(8):
                nc.tensor.matmul(gp[:], wg[:, kk, :], outTs[:, kk, :],
                                 start=(kk == 0), stop=(kk == 7))
            gt = spool.tile([128, LBLK], F32, tag="gt", name=f"gt{g}")
            nc.scalar.activation(gt[:], gp[:], AF.Sigmoid,
                                 bias=biases["bge"][:, g:g + 1])
            d1 = spool.tile([128, LBLK], F32, tag="fin", name=f"d1{g}")
            nc.vector.tensor_sub(d1[:], pj[:], xrs[:, g, :])
            d2 = spool.tile([128, LBLK], F32, tag="fin", name=f"d2{g}")
            nc.vector.tensor_mul(d2[:], d1[:], gt[:])
            fo = spool.tile([128, LBLK], F32, tag="fin", name=f"fo{g}")
            nc.vector.tensor_add(fo[:], d2[:], xrs[:, g, :])
            nc.sync.dma_start(OUTT.ap()[g * 128:(g + 1) * 128, :], fo[:])

    nc.compile()
    return nc


def kernel(x, kg_embeds, Wq, bq, Wk, bk, Wv, bv, Wkk, bkk, Wkv, bkv,
           Wo, bo, Wg, bg):
    import ml_dtypes
    bf16 = ml_dtypes.bfloat16

    x = np.asarray(x, np.float32)
    kg_embeds = np.asarray(kg_embeds, np.float32)
    ws = {k: np.asarray(v, np.float32) for k, v in dict(
        Wq=Wq, bq=bq, Wk=Wk, bk=bk, Wv=Wv, bv=bv, Wkk=Wkk, bkk=bkk,
        Wkv=Wkv, bkv=bkv, Wo=Wo, bo=bo, Wg=Wg, bg=bg).items()}

    if "nc" not in _CACHE:
        _CACHE["nc"] = _build()
    nc = _CACHE["nc"]

    def col8(v):
        return np.ascontiguousarray(v.reshape(8, 128).T)

    shared = {
        "WqT": np.ascontiguousarray(ws["Wq"].T.astype(bf16)),
        "WkT": np.ascontiguousarray(ws["Wk"].T.astype(bf16)),
        "WkkT": np.ascontiguousarray(ws["Wkk"].T.astype(bf16)),
        "WvT": np.ascontiguousarray(ws["Wv"].T.astype(bf16)),
        "WkvT": np.ascontiguousarray(ws["Wkv"].T.astype(bf16)),
        "WoT": np.ascontiguousarray(ws["Wo"].T.astype(bf16)),
        "WgT": np.ascontiguousarray(ws["Wg"][:, :D].T.astype(bf16)),
        "bq": col8(ws["bq"] * 0.125),
        "bk": col8(ws["bk"]),
        "bkk": col8(ws["bkk"]),
        "bo": col8(ws["bo"]),
        "bvb": np.ascontiguousarray(np.tile(ws["bv"], (128, 1))),
        "bkvb": np.ascontiguousarray(np.tile(ws["bkv"], (128, 1))),
    }

    in_maps = []
    for c in range(N_CORES):
        b, j = divmod(c, 4)
        # roll the core's query block to columns [0, 512); k/v attend over
        # all columns, so their (rolled) order is irrelevant to softmax
        xb = np.ascontiguousarray(np.roll(x[b].T, -j * LBLK, axis=1))
        kgm = kg_embeds[b].mean(axis=0)
        bge = ws["bg"] + ws["Wg"][:, D:] @ kgm
        m = dict(shared)
        m["xT"] = xb.astype(bf16)
        m["xres"] = np.ascontiguousarray(
            xb[:, :LBLK].reshape(8, 128, LBLK).transpose(1, 0, 2))
        m["kgT"] = np.ascontiguousarray(kg_embeds[b].T.astype(bf16))
        m["bge"] = col8(bge)
        in_maps.append(m)

    _CACHE["in_maps"] = in_maps
    res = run_bass_kernel_spmd(nc, in_maps, core_ids=list(range(N_CORES)))
    out = np.empty((B, L, D), np.float32)
    for c in range(N_CORES):
        b, j = divmod(c, 4)
        out[b, j * LBLK:(j + 1) * LBLK, :] = res.results[c]["OUTT"].T
    return out
